# revision 11
# baseline (speedup 1.0000x reference)
import sys

sys.path.insert(0, "/opt/trn_rl_repo")

import numpy as np
import ml_dtypes

import concourse.bacc as bacc
import concourse.bass as bass
import concourse.mybir as mybir
import concourse.tile as tile
from concourse.bass_utils import run_bass_kernel_spmd

F32 = mybir.dt.float32
BF16 = mybir.dt.bfloat16
FP8 = mybir.dt.float8e4
AF = mybir.ActivationFunctionType
ALU = mybir.AluOpType
AX = mybir.AxisListType
PM = mybir.MatmulPerfMode

NPF8 = ml_dtypes.float8_e4m3
NPBF = ml_dtypes.bfloat16

# Problem constants (hardcoded per harness contract).
B, C, H, W = 4, 64, 128, 128
NT = 9          # 3x3 taps
NCORES = 8
HL = H // 2     # 64 coarse rows per core
NYB = 8         # y-blocks
YB = HL // NYB  # 8 rows per block
WR = YB + 2     # conv window rows per block (halo)
NP = YB * 64    # 512: elems per (tap, ff) slab per partition
PB = NT * 4 * NP  # 18432: prod tile free size

_cached = {}


def ap_of(t, off, dims):
    base = t[:]
    return bass.AP(base.tensor, base.offset + off, dims)


def build_nc():
    nc = bacc.Bacc("TRN2", target_bir_lowering=False, debug=False, num_devices=NCORES)

    hp2_d = nc.dram_tensor("hp2", [128, 66 * 130 + 4], FP8, kind="ExternalInput")
    hps_d = nc.dram_tensor("hps", [128, 66 * 130], FP8, kind="ExternalInput")
    hT9_d = nc.dram_tensor("hT9", [128, NT * 64 * 64], BF16, kind="ExternalInput")
    w1a_d = nc.dram_tensor("w1a", [128, 256], FP8, kind="ExternalInput")
    w1b_d = nc.dram_tensor("w1b", [128, 256], FP8, kind="ExternalInput")
    w1c_d = nc.dram_tensor("w1c", [128, 128], FP8, kind="ExternalInput")
    b1_d = nc.dram_tensor("b1c", [128, 1], F32, kind="ExternalInput")
    w2t_d = nc.dram_tensor("w2t", [128, 36], BF16, kind="ExternalInput")
    one_d = nc.dram_tensor("onec", [1, 128], BF16, kind="ExternalInput")
    b2r_d = nc.dram_tensor("b2r", [1, YB * 36], BF16, kind="ExternalInput")
    idb_d = nc.dram_tensor("idb", [128, 128], BF16, kind="ExternalInput")
    out_d = nc.dram_tensor("out", [64, H, 2 * W], BF16, kind="ExternalOutput")

    with tile.TileContext(nc) as tc:
        with (
            tc.tile_pool(name="const", bufs=1) as cpool,
            tc.tile_pool(name="ring", bufs=2) as ring,
            tc.tile_pool(name="mchunk", bufs=2) as mpool,
            tc.tile_pool(name="masks", bufs=2) as kpool,
            tc.tile_pool(name="prodp", bufs=2) as ppool,
            tc.tile_pool(name="accp", bufs=2) as apool,
            tc.tile_pool(name="trp", bufs=2) as tpool,
            tc.tile_pool(name="orow", bufs=2) as opool,
            tc.tile_pool(name="ps1", bufs=2, space=bass.MemorySpace.PSUM) as pp1,
            tc.tile_pool(name="pst", bufs=2, space=bass.MemorySpace.PSUM) as ppt,
            tc.tile_pool(name="psa", bufs=2, space=bass.MemorySpace.PSUM) as ppa,
        ):
            # ---- constants ----
            w1a = cpool.tile([128, 256], FP8)
            w1b = cpool.tile([128, 256], FP8)
            w1c = cpool.tile([128, 128], FP8)
            b1 = cpool.tile([128, 1], F32)
            w2t = cpool.tile([128, 36], BF16)
            onec = cpool.tile([1, 128], BF16)
            b2r = cpool.tile([1, YB * 36], BF16)
            idb = cpool.tile([128, 128], BF16)
            nc.sync.dma_start(w1a[:], w1a_d[:])
            nc.sync.dma_start(w1b[:], w1b_d[:])
            nc.sync.dma_start(w1c[:], w1c_d[:])
            nc.sync.dma_start(b1[:], b1_d[:])

            def late_consts():
                nc.sync.dma_start(w2t[:], w2t_d[:])
                nc.sync.dma_start(onec[:], one_d[:])
                nc.sync.dma_start(b2r[:], b2r_d[:])
                nc.sync.dma_start(idb[:], idb_d[:])
                # warm the ACT function tables before the pipeline needs them
                warm = cpool.tile([128, 1], F32)
                nc.vector.memset(warm[:], 0.0)
                nc.scalar.activation(warm[:], warm[:], AF.Exp)

            def dma_in(yb):
                r0 = yb * YB
                hp2b = ring.tile([128, WR * 130 + 4], FP8, tag="hp2b")
                hpsb = ring.tile([128, WR * 130], FP8, tag="hpsb")
                hT9b = ring.tile([128, NT * NP], BF16, tag="hT9b")
                nc.sync.dma_start(hp2b[:], hp2_d[:, r0 * 130:r0 * 130 + WR * 130 + 4])
                nc.sync.dma_start(hpsb[:], hps_d[:, r0 * 130:(r0 + WR) * 130])
                nc.sync.dma_start(
                    hT9b[:],
                    ap_of(hT9_d, r0 * 64,
                          [[NT * 64 * 64, 128], [64 * 64, NT], [1, NP]]))
                return hp2b, hpsb, hT9b

            def conv(yb, bufs):
                """conv1 -> relu via fp8 DoubleRow paired-tap matmuls."""
                hp2b, hpsb, hT9b = bufs
                m1 = mpool.tile([128, 2 * 512], BF16, tag="m1")
                for ic in range(2):
                    r = 4 * ic
                    ps1 = pp1.tile([128, 512], F32)
                    # A: ktiles (t00,t01)+(t11,t12) via hp2, offset delta 131
                    rhs = ap_of(hp2b, r * 130,
                                [[WR * 130 + 4, 128], [131, 2], [130, 4], [1, 128]])
                    lhs = ap_of(w1a, 0, [[256, 128], [128, 2], [1, 128]])
                    nc.tensor.matmul(ps1[:], lhs, rhs, start=True, stop=False,
                                     perf_mode=PM.DoubleRow)
                    # B: ktiles (t20,t21)+(t22,0) via hp2, offset delta 2
                    rhs = ap_of(hp2b, (r + 2) * 130,
                                [[WR * 130 + 4, 128], [2, 2], [130, 4], [1, 128]])
                    lhs = ap_of(w1b, 0, [[256, 128], [128, 2], [1, 128]])
                    nc.tensor.matmul(ps1[:], lhs, rhs, start=False, stop=False,
                                     perf_mode=PM.DoubleRow)
                    # C: pair (t02,t10) via hps, plain fp8
                    rhs = ap_of(hpsb, r * 130 + 2,
                                [[WR * 130, 128], [130, 4], [1, 128]])
                    nc.tensor.matmul(ps1[:], w1c[:], rhs, start=False, stop=True)
                    nc.scalar.activation(m1[:, ic * 512:(ic + 1) * 512], ps1[:],
                                         AF.Relu, bias=b1[:], scale=1.0)
                return m1

            def masks(yb, m1):
                """conv2 (+bias) -> exp -> Z -> 1/Z -> masks nm[x; ff,t,y,2]."""
                pst = ppt.tile([128, YB * 36], F32)
                nc.tensor.matmul(pst[:], onec[:], b2r[:],
                                 start=True, stop=False, skip_group_check=True)
                for yl in range(YB):
                    nc.tensor.matmul(pst[:, yl * 36:(yl + 1) * 36],
                                     m1[:, yl * 128:(yl + 1) * 128], w2t[:],
                                     start=False, stop=(yl == YB - 1),
                                     skip_group_check=True)
                eT = kpool.tile([128, YB * 36], BF16, tag="eT")
                nc.scalar.activation(eT[:], pst[:], AF.Exp)

                zb = kpool.tile([128, YB * 4], F32, tag="zb")
                rz = kpool.tile([128, YB * 4], F32, tag="rz")
                z_in = ap_of(eT, 0, [[YB * 36, 128], [36, YB], [9, 4], [1, 9]])
                z_out = ap_of(zb, 0, [[YB * 4, 128], [4, YB], [1, 4]])
                nc.vector.tensor_reduce(z_out, z_in, AX.X, ALU.add)
                nc.vector.reciprocal(rz[:], zb[:])

                # nm[x; ff*144 + t*16 + y*2 + pair] = softmax mask, pair-duplicated
                nm = kpool.tile([128, 656], BF16, tag="nm")
                i0 = ap_of(eT, 0, [[YB * 36, 128], [9, 4], [36, YB], [1, 9]])
                i1 = ap_of(rz, 0, [[YB * 4, 128], [1, 4], [4, YB], [0, 9]])
                for pair in range(2):
                    o_ap = ap_of(nm, pair, [[656, 128], [144, 4], [2, YB], [16, 9]])
                    nc.gpsimd.tensor_tensor(o_ap, i0, i1, ALU.mult)
                return nm

            def units(yb, state):
                """products + tap-sum + evict + transpose + interleave + out."""
                hT9b, nm = state
                # prod layout [x; fy(9216) t(1024) y(128) fx(64) c(1)]
                prod = ppool.tile([128, PB + 2304], BF16, tag="prod")
                # products: one DVE op per ff over fused (y,t) and split c
                i0 = ap_of(hT9b, 0,
                           [[NT * NP, 128], [64, 72], [2, 32], [1, 2]])
                for ff in range(4):
                    fy, fx = ff // 2, ff % 2
                    po = ap_of(prod, fy * 9216 + fx * 64,
                               [[PB + 2304, 128], [128, 72], [2, 32], [1, 2]])
                    i1 = ap_of(nm, ff * 144,
                               [[656, 128], [2, 72], [0, 32], [1, 2]])
                    nc.vector.tensor_tensor(po, i0, i1, ALU.mult)

                sA = apool.tile([128, 2304], BF16, tag="sA")
                for fy in range(2):
                    # psA[x; y*128 + fx*64 + c]
                    psA = ppa.tile([128, 1024], F32)
                    for h in range(2):
                        for t in range(NT):
                            rhs = ap_of(prod, fy * 9216 + t * 1024 + h * 512,
                                        [[PB + 2304, 128], [1, 512]])
                            nc.tensor.matmul(psA[:, h * 512:(h + 1) * 512],
                                             idb[:], rhs,
                                             start=(t == 0), stop=(t == NT - 1),
                                             skip_group_check=True)
                    # evict f32->bf16 with slab interleave:
                    # sA[x, (4j+2fy+fx)*128 + 2c + y2] = psA[x, (2j+y2)*128+fx*64+c]
                    o_ap = ap_of(sA, fy * 256,
                                 [[2304, 128], [512, 4], [1, 2], [2, 128]])
                    i_ap = ap_of(psA, 0,
                                 [[1024, 128], [256, 4], [128, 2], [1, 128]])
                    nc.scalar.copy(o_ap, i_ap)

                # one chunked transpose: tr[s, ch*128+x] = sA[x, ch*128+s]
                tr = tpool.tile([128, 2304], BF16, tag="tr")
                t_out = ap_of(tr, 0, [[2304, 128], [128, 16], [1, 128]])
                nc.sync.dma_start_transpose(t_out, sA[:, 0:2048])

                # fx interleave: orow[s, j*512+fy*256+2x+fx] = tr[s, (4j+2fy+fx)*128+x]
                orow = opool.tile([128, 2304], BF16, tag="orow")
                for fy in range(2):
                    i_ap = ap_of(tr, fy * 256,
                                 [[2304, 128], [128, 2], [512, 4], [1, 128]])
                    o_ap = ap_of(orow, fy * 256,
                                 [[2304, 128], [1, 2], [512, 4], [2, 128]])
                    nc.scalar.copy(o_ap, i_ap)

                # out rows 16*yb+4j .. +3 ; partitions (c, y2), free (fy, 2x+fx)
                for j in range(4):
                    dst = ap_of(out_d, (16 * yb + 4 * j) * 256,
                                [[H * 256, 64], [512, 2], [1, 512]])
                    nc.sync.dma_start(dst, orow[:, j * 512:(j + 1) * 512])

            # software pipeline, one block lag for the units stage
            bufs = dma_in(0)
            late_consts()
            eb = conv(0, bufs)
            prev = (bufs[2], masks(0, eb))
            for yb in range(1, NYB):
                bufs = dma_in(yb)
                eb = conv(yb, bufs)
                units(yb - 1, prev)
                prev = (bufs[2], masks(yb, eb))
            units(NYB - 1, prev)

    nc.compile()
    return nc


def prep_shared(W1, b1, W2, b2):
    W1 = np.asarray(W1, np.float32)
    b1 = np.asarray(b1, np.float32)
    W2 = np.asarray(W2, np.float32).reshape(36, 128)
    b2 = np.asarray(b2, np.float32)

    def wslab(dy, dx):
        return np.ascontiguousarray(W1[:, :, dy, dx].T)  # [cin=64, cout=128]

    w1a = np.zeros((128, 256), np.float32)
    w1a[0:64, 0:128] = wslab(0, 0)
    w1a[64:128, 0:128] = wslab(0, 1)
    w1a[0:64, 128:256] = wslab(1, 1)
    w1a[64:128, 128:256] = wslab(1, 2)
    w1b = np.zeros((128, 256), np.float32)
    w1b[0:64, 0:128] = wslab(2, 0)
    w1b[64:128, 0:128] = wslab(2, 1)
    w1b[0:64, 128:256] = wslab(2, 2)
    w1c = np.zeros((128, 128), np.float32)
    w1c[0:64] = wslab(0, 2)
    w1c[64:128] = wslab(1, 0)

    # w2t col k = (ff = k//9, t = k%9) <- original channel t*4+ff
    o_of_mp = np.array([t * 4 + ff for ff in range(4) for t in range(9)])
    w2t = np.ascontiguousarray((0.25 * W2[o_of_mp, :]).T)
    b2c = np.ascontiguousarray((0.25 * b2[o_of_mp]).reshape(36, 1))

    return {
        "w1a": w1a.astype(NPF8),
        "w1b": w1b.astype(NPF8),
        "w1c": w1c.astype(NPF8),
        "b1c": b1.reshape(128, 1).astype(np.float32),
        "w2t": w2t.astype(NPBF),
        "onec": np.ones((1, 128), dtype=NPBF),
        "b2r": np.tile(b2c.ravel(), YB).reshape(1, -1).astype(NPBF),
        "idb": np.eye(128, dtype=NPBF),
    }


def kernel(h, W1, b1, W2, b2, _trace=False):
    h = np.asarray(h, np.float32)
    shared = prep_shared(W1, b1, W2, b2)

    hp = np.pad(h, ((0, 0), (0, 0), (1, 1), (1, 1)))  # [B, C, 130, 130]
    in_maps = []
    for core in range(NCORES):
        b, half = core // 2, core % 2
        y0 = half * HL
        win = hp[b, :, y0:y0 + 66, :]  # [64, 66, 130] f32
        winf = win.reshape(64, -1)
        hp2 = np.zeros((128, 66 * 130 + 4), np.float32)
        hp2[0:64, 0:66 * 130] = winf
        hp2[64:128, 0:66 * 130 - 1] = winf[:, 1:]
        hps = np.zeros((128, 66 * 130), np.float32)
        hps[0:64] = winf
        hps[64:128, 0:66 * 130 - 128] = winf[:, 128:]
        w8 = 8.0 * win
        # hT9[x, t*4096 + yy*64 + c] = w8[c, yy+dy, x+dx]
        hT9 = np.empty((128, NT, 64, 64), np.float32)
        for dy in range(3):
            for dx in range(3):
                t = dy * 3 + dx
                # w8[:, dy:dy+64, dx:dx+128] -> [x, yy, c]
                hT9[:, t] = w8[:, dy:dy + 64, dx:dx + 128].transpose(2, 1, 0)
        m = dict(shared)
        m["hp2"] = hp2.astype(NPF8)
        m["hps"] = hps.astype(NPF8)
        m["hT9"] = hT9.reshape(128, -1).astype(NPBF)
        in_maps.append(m)

    if "nc" not in _cached:
        _cached["nc"] = build_nc()
    res = run_bass_kernel_spmd(_cached["nc"], in_maps, core_ids=list(range(NCORES)),
                               trace=_trace)

    out = np.zeros((B, C, 2 * H, 2 * W), np.float32)
    for core in range(NCORES):
        b, half = core // 2, core % 2
        out[b, :, half * 128:(half + 1) * 128, :] = np.asarray(
            res.results[core]["out"], dtype=np.float32)
    if _trace:
        return out, res
    return out
